# revision 1
# baseline (speedup 1.0000x reference)
"""Trainium2 Bass kernel: depthwise transposed-conv2d (4x bilinear upsampling).

Math: out = conv_transpose2d(x, W, stride=4), W = 7x7 bilinear kernel per
channel (depthwise, 256 channels). In: [4,256,64,64] f32 -> out [4,256,259,259].

The bilinear kernel is separable (v = [1,2,3,4,3,2,1]/4 outer product) and the
transposed conv decomposes into 4 polyphase streams per axis:
    out1d[4q+s] = x[q-1] + b_s*(x[q] - x[q-1]),  b = (0.25, 0.5, 0.75),  s=0..2
    out1d[4q+3] = x[q]
with x[-1] = x[64] = 0 (so out1d has 259 = 3*65 + 64 entries).

Sharding: pure data parallel. N*C = 1024 (n,c) slices, 128 per core on 8
cores; each slice is one SBUF partition (its 64x64 image in the free dim).

Per-core pipeline (all per-partition, raw Bass, manual semaphores):
  1. DMA-in x -> xt [64 rows, 66 cols] (zero col pads).
  2. DVE: D1 = xt[:,1:] - xt[:,:-1]; 3x scalar_tensor_tensor writes the three
     W-phases strided (step 4) into X1p; ACT copies phase-3 (pure copy).
     X1p = [65 rows, 259]: row 0 = zero pad, rows 1..64 = W-upsampled rows.
  3. Per band b (8 q-values -> 32 consecutive output rows, 8 bands):
     GPSIMD: D2 = X1p[q+1]-X1p[q]; DVE: 3 STT phase rows; ACT: phase-3 row
     copies -- assembled interleaved in a band tile so DMA-out is one fully
     contiguous 33KB/partition write.
  4. Tail rows 256..258 = (1-b_s) * X1p[64] via ACT scaled copies.
"""

import numpy as np

N, C, H, W = 4, 256, 64, 64
RATE = 4
OW = (W - 1) * RATE + 7  # 259
P = 128          # partitions per core = images per core
NCORES = 8

XT_W = W + 2          # 66: zero col, 64 data cols, zero col
XT_N = H * XT_W       # 4224
X1_R = H + 1          # 65: zero pad row + 64 data rows
X1_N = X1_R * OW      # 16835
D1_N = H * (W + 1)    # 64*65
QB = 8                # q-values per band
NBAND = 8             # 8*8 = 64 q-values in full bands; q=64 handled in tail
D2_N = QB * OW        # 2072
BAND_N = 4 * QB * OW  # 8288 = 32 output rows
TAIL_N = 3 * OW       # 777

_CACHE = {}


def _build_nc():
    import concourse.bass as bass
    import concourse.mybir as mybir

    f32 = mybir.dt.float32
    add = mybir.AluOpType.add
    mult = mybir.AluOpType.mult
    sub = mybir.AluOpType.subtract

    nc = bass.Bass()
    x = nc.declare_dram_parameter("x", [P, H, W], f32, isOutput=False)
    out = nc.declare_dram_parameter("out", [P, OW, OW], f32, isOutput=True)

    xf = x.rearrange("p h w -> p (h w)")      # [128, 4096]
    of = out.rearrange("p h w -> p (h w)")    # [128, 67081]

    BS = (0.25, 0.5, 0.75)   # b_s for phases 0..2
    AS = (0.75, 0.5, 0.25)   # tail scales (1 - b_s)

    def v(t, off, dims):
        """Strided view of a flat [128, N] sbuf tensor."""
        full = t[:]
        return bass.AP(full.tensor, off, [list(full.ap[0])] + [list(d) for d in dims])

    with (
        nc.sbuf_tensor([P, XT_N], f32) as xt,
        nc.sbuf_tensor([P, X1_N], f32) as x1p,
        nc.sbuf_tensor([P, D1_N], f32) as d1,
        nc.sbuf_tensor([P, D2_N], f32) as d2a,
        nc.sbuf_tensor([P, D2_N], f32) as d2b,
        nc.sbuf_tensor([P, BAND_N], f32) as bda,
        nc.sbuf_tensor([P, BAND_N], f32) as bdb,
        nc.semaphore("dma_in") as dma_in,
        nc.semaphore("dma_out") as dma_out,
        nc.semaphore("s_gp") as s_gp,
        nc.semaphore("s_x1v") as s_x1v,
        nc.semaphore("s_x1a") as s_x1a,
        nc.semaphore("s_d2") as s_d2,
        nc.semaphore("s_dveb") as s_dveb,
        nc.semaphore("s_actb") as s_actb,
        nc.Block() as block,
    ):
        d2t = (d2a, d2b)
        bdt = (bda, bdb)

        @block.sync
        def _(sync):
            # load x into xt cols 1..64 of each 66-wide row
            sync.dma_start(
                out=v(xt, 1, [[XT_W, H], [1, W]]),
                in_=bass.AP(xf.tensor, 0, [list(xf.ap[0]), [W, H], [1, W]]),
            ).then_inc(dma_in, 16)
            for b in range(NBAND):
                sync.wait_ge(s_dveb, b + 1)
                sync.wait_ge(s_actb, b + 1)
                o0 = 4 * QB * b * OW
                sync.dma_start(
                    out=of[:, o0:o0 + BAND_N], in_=bdt[b % 2][:]
                ).then_inc(dma_out, 16)
            sync.wait_ge(s_actb, NBAND + 1)
            sync.dma_start(
                out=of[:, 256 * OW:], in_=bda[:, :TAIL_N]
            ).then_inc(dma_out, 16)
            sync.wait_ge(dma_out, (NBAND + 1) * 16)

        @block.vector
        def _(vector):
            vector.wait_ge(dma_in, 16)
            vector.wait_ge(s_gp, 1)
            # D1[r, q] = xt[r, q+1] - xt[r, q]  (q = 0..64 over 66-wide rows)
            vector.tensor_tensor(
                out=v(d1, 0, [[W + 1, H], [1, W + 1]]),
                in0=v(xt, 1, [[XT_W, H], [1, W + 1]]),
                in1=v(xt, 0, [[XT_W, H], [1, W + 1]]),
                op=sub,
            )
            # W-phases: X1p[1+r, 4q+s] = xt[r, q] + b_s * D1[r, q]
            for s in range(3):
                ins = vector.scalar_tensor_tensor(
                    out=v(x1p, OW + s, [[OW, H], [4, W + 1]]),
                    in0=v(d1, 0, [[W + 1, H], [1, W + 1]]),
                    scalar=BS[s],
                    in1=v(xt, 0, [[XT_W, H], [1, W + 1]]),
                    op0=mult,
                    op1=add,
                )
                if s == 2:
                    ins.then_inc(s_x1v, 1)
            # bands: band rows 4j+s = X1p[q0+j] + b_s * D2[j]
            for b in range(NBAND):
                vector.wait_ge(s_d2, b + 1)
                if b >= 2:
                    vector.wait_ge(dma_out, (b - 1) * 16)
                q0 = QB * b
                for s in range(3):
                    ins = vector.scalar_tensor_tensor(
                        out=v(bdt[b % 2], s * OW, [[4 * OW, QB], [1, OW]]),
                        in0=v(d2t[b % 2], 0, [[OW, QB], [1, OW]]),
                        scalar=BS[s],
                        in1=v(x1p, q0 * OW, [[OW, QB], [1, OW]]),
                        op0=mult,
                        op1=add,
                    )
                    if s == 2:
                        ins.then_inc(s_dveb, 1)

        @block.scalar
        def _(scalar):
            scalar.wait_ge(dma_in, 16)
            # W-phase 3 (pure copy): X1p[1+r, 4m+3] = xt[r, m+1], m = 0..63
            scalar.copy(
                out=v(x1p, OW + 3, [[OW, H], [4, W]]),
                in_=v(xt, 1, [[XT_W, H], [1, W]]),
            ).then_inc(s_x1a, 1)
            scalar.wait_ge(s_x1v, 1)
            for b in range(NBAND):
                if b >= 2:
                    scalar.wait_ge(dma_out, (b - 1) * 16)
                q0 = QB * b
                # band rows 4j+3 = X1p[q0+j+1]
                scalar.copy(
                    out=v(bdt[b % 2], 3 * OW, [[4 * OW, QB], [1, OW]]),
                    in_=v(x1p, (q0 + 1) * OW, [[OW, QB], [1, OW]]),
                ).then_inc(s_actb, 1)
            # tail: out rows 256+s = (1-b_s) * X1p[64]
            scalar.wait_ge(dma_out, (NBAND - 1) * 16)
            for s in range(3):
                ins = scalar.mul(
                    out=v(bda, s * OW, [[OW, 1], [1, OW]]),
                    in_=v(x1p, H * OW, [[OW, 1], [1, OW]]),
                    mul=AS[s],
                )
                if s == 2:
                    ins.then_inc(s_actb, 1)

        @block.gpsimd
        def _(gpsimd):
            # zero pads: xt cols 0 and 65; X1p row 0
            gpsimd.memset(v(xt, 0, [[XT_W, H], [W + 1, 2]]), 0.0).then_inc(s_gp, 1)
            gpsimd.memset(v(x1p, 0, [[OW, 1], [1, OW]]), 0.0)
            gpsimd.wait_ge(s_x1v, 1)
            gpsimd.wait_ge(s_x1a, 1)
            for b in range(NBAND):
                if b >= 2:
                    gpsimd.wait_ge(s_dveb, b - 1)
                q0 = QB * b
                gpsimd.tensor_tensor(
                    out=v(d2t[b % 2], 0, [[OW, QB], [1, OW]]),
                    in0=v(x1p, (q0 + 1) * OW, [[OW, QB], [1, OW]]),
                    in1=v(x1p, q0 * OW, [[OW, QB], [1, OW]]),
                    op=sub,
                ).then_inc(s_d2, 1)

    return nc


def kernel(x: np.ndarray, weight: np.ndarray | None = None) -> np.ndarray:
    from concourse.bass_utils import run_bass_kernel_spmd

    if "nc" not in _CACHE:
        _CACHE["nc"] = _build_nc()
    nc = _CACHE["nc"]

    xs = np.ascontiguousarray(x, dtype=np.float32).reshape(N * C, H, W)
    core_ids = list(range(NCORES))
    in_maps = [{"x": xs[i * P:(i + 1) * P]} for i in core_ids]
    res = run_bass_kernel_spmd(nc, in_maps, core_ids)
    outs = np.stack([res.results[i]["out"] for i in core_ids])  # [8,128,259,259]
    return outs.reshape(N, C, OW, OW)
